# revision 1
# baseline (speedup 1.0000x reference)
"""Trainium2 Bass kernel for CausalWanSelfAttention (frame-causal windowed
attention with QK-RMSNorm + RoPE), sharded over 8 NeuronCores.

Sharding: each core owns T = (h*w)/8 tokens of every frame (frame-balanced
interleave).  Each core computes Q/K/V projections + RMSNorm + RoPE for its
own tokens; K and V are exchanged in ONE merged AllGather (collective
bandwidth improves with transfer size), which overlaps the Q projection +
RoPE; attention + O-projection are computed locally for the core's queries.

Device layouts / tricks:
  - q/k feature-major [ch, tok] (channels on partitions), with each head's
    128 channels permuted to [re(0..63) | im(0..63)] so RoPE works on
    contiguous partition blocks (permutation is folded into Wq/Wk on host).
  - RoPE half-swap runs on the PE (host-supplied permutation matrix);
    SBUF-to-SBUF swap DMAs would serialize against the collective.
  - v token-major [tok, ch]; gathered keys are re-chunked ACROSS cores into
    13 x 120-key chunks per frame (no 67-wide runt matmuls).
  - scores s^T [keys, q]; per (head, chunk) ONE [kw, 951] score tile covers
    all key frames' q<512 range and ONE [kw, 219] tile the q>=512 tail, so
    exp is 2 ACT instructions per chunk (ACT is the attention bottleneck).
  - softmax denominator: exp tiles are summed on the DVE (bf16 running sum);
    Z = 6 ones-matmuls per head in partition-broadcast form, so 1/Z is a
    direct DVE reciprocal -- no DRAM broadcast roundtrip.
  - RMSNorm scale r (per token) is folded into the RoPE cos/sin tables
    (scalar multiplication commutes with rotation); per-channel gain g and
    bias b are folded into the ACT eviction (per-partition scale/bias).
  - weight DMAs are staged early/split so the PE never waits on the DMA
    queue behind compute-dependent transfers.
"""

import math
import sys
from contextlib import ExitStack

import numpy as np

if "/opt/trn_rl_repo" not in sys.path:
    sys.path.insert(0, "/opt/trn_rl_repo")

import ml_dtypes

BF16 = ml_dtypes.bfloat16
NC = 8  # cores
D = 128  # head dim
EPS = 1e-6


# ---------------------------------------------------------------------------
# helpers
# ---------------------------------------------------------------------------
def _pieces(lo, hi, T):
    """Split the global (within-frame) token range [lo, hi) into per-core
    pieces.  Returns [(core, a, b)] with a/b local to the core's frame-chunk."""
    out = []
    c = lo // T
    while lo < hi:
        b = min(hi, (c + 1) * T)
        out.append((c, lo - c * T, b - c * T))
        lo = b
        c += 1
    return out


def _segs(q0, S, bank=512):
    """Split [q0, S) at multiples of `bank` -> list of absolute (qa, qb)."""
    pts = [q0]
    nxt = (q0 // bank + 1) * bank
    while nxt < S:
        pts.append(nxt)
        nxt += bank
    pts.append(S)
    return [(pts[i], pts[i + 1]) for i in range(len(pts) - 1)]


def _chunks(frame_len, width=128):
    return [(g * width, min(frame_len, (g + 1) * width))
            for g in range((frame_len + width - 1) // width)]


# ---------------------------------------------------------------------------
# device program
# ---------------------------------------------------------------------------
_BUILD_CACHE = {}


def build_program(NH, F, T, allowed_kf, cap_waits=True):
    """Build the SPMD Bass program (identical on all 8 cores).

    NH: number of heads; F: frames; T: tokens per (core, frame);
    allowed_kf[qf] = list of key frames query-frame qf may attend to
    (must make, for each kf, the attending q-set a contiguous suffix of
    frames -- true for causal masks).
    """
    key = (NH, F, T, tuple(tuple(a) for a in allowed_kf), cap_waits)
    if key in _BUILD_CACHE:
        return _BUILD_CACHE[key]

    import concourse.bass as bass
    import concourse.mybir as mybir
    import concourse.tile as tile
    from concourse.mybir import ActivationFunctionType as AF

    dt = mybir.dt
    DIM = NH * D
    S = F * T              # tokens per core
    FRAME = NC * T         # tokens per frame
    NHALF = 2
    H0 = (S + 1) // 2      # token halves for the q/k projections
    SLICE = min(512, DIM)  # out-channel slice for v/o projections
    NSL = DIM // SLICE
    TOKCH = _chunks(S, 128)  # token chunks for v/o projections

    # for each key frame kf: the first query frame that attends to it, and
    # check the q-set is a suffix
    first_qf = {}
    for kf in range(F):
        qs = [qf for qf in range(F) if kf in allowed_kf[qf]]
        assert qs, f"key frame {kf} unused"
        assert qs == list(range(qs[0], F)), "non-suffix q-set unsupported"
        first_qf[kf] = qs[0]

    nc = bass.Bass()

    # ---------------- I/O ----------------
    xT_d = nc.dram_tensor("xT", [DIM, S], dt.bfloat16, kind="ExternalInput")
    w_d = {}
    for nm in ("wqT", "wkT", "wvT", "woT"):
        w_d[nm] = nc.dram_tensor(nm, [DIM, DIM], dt.bfloat16, kind="ExternalInput")
    # packed per-channel affine constants: bq|gq|bq*gq|bk|gk|bk*gk
    bias_d = nc.dram_tensor("bias_pack", [128, 6 * NH], dt.float32,
                            kind="ExternalInput")
    swp_d = nc.dram_tensor("swp", [128, 128], dt.bfloat16,
                           kind="ExternalInput")
    bv_d = nc.dram_tensor("bv_r", [1, DIM], dt.bfloat16, kind="ExternalInput")
    bo_d = nc.dram_tensor("bo_r", [1, DIM], dt.float32, kind="ExternalInput")
    angS_d = nc.dram_tensor("angS", [128, S], dt.float32, kind="ExternalInput")
    angC_d = nc.dram_tensor("angC", [128, S], dt.float32, kind="ExternalInput")
    out_d = nc.dram_tensor("out", [S, DIM], dt.float32, kind="ExternalOutput")

    rg = [list(range(NC))]
    inv_sqrt_d = 1.0 / math.sqrt(D)

    with tile.TileContext(nc) as tc, ExitStack() as ctx:
        dram = ctx.enter_context(tc.tile_pool(name="dram", bufs=1, space="DRAM"))
        # merged K|V exchange buffer: rows 0:DIM = k^T [DIM, S]; the flat tail
        # DIM*S: holds v [S, DIM] row-major.  One big AllGather beats two
        # (collective bandwidth improves with transfer size).
        kv_loc = dram.tile([2 * DIM, S], dt.bfloat16)
        kv_all = dram.tile([NC * 2 * DIM, S], dt.bfloat16, addr_space="Shared")
        kv_base = kv_loc[:, :].offset
        kva_base = kv_all[:, :].offset

        const = ctx.enter_context(tc.tile_pool(name="const", bufs=1))
        resid = ctx.enter_context(tc.tile_pool(name="resid", bufs=1))
        # created up-front so pg0's K gathers can be issued right after the
        # AllGather, ahead of everything else in the DMA queue
        att_k = ctx.enter_context(tc.tile_pool(name="att_k", bufs=7))

        def stage_kr(pg):
            out = {}
            for kf in range(F):
                out[kf] = att_k.tile([128, 2, NC * T], dt.bfloat16,
                                     tag="kr", name=f"kr{kf}")
            # hi-outer issue order: the first head's three frames land first,
            # so its scores/exp start ~4us sooner
            for hi in range(2):
                for kf in range(F):
                    nc.sync.dma_start(
                        out=out[kf][:, hi, :]
                        .rearrange("p (c t) -> p c t", c=NC),
                        in_=bass.AP(
                            tensor=kv_all.tensor,
                            offset=kva_base + (pg * 256 + hi * 128) * S
                            + kf * T,
                            ap=[[S, 128], [2 * DIM * S, NC], [1, T]]))
            return out

        ones_key = const.tile([128, 1], dt.bfloat16)
        nc.vector.memset(ones_key, 1.0)
        ones_row = const.tile([1, 128], dt.bfloat16)
        nc.vector.memset(ones_row, 1.0)
        ones128 = const.tile([128, 128], dt.bfloat16)
        nc.vector.memset(ones128, 1.0)
        ones_rowf = const.tile([1, 128], dt.float32)
        nc.vector.memset(ones_rowf, 1.0)
        zeros_sb = const.tile([128, 512], dt.bfloat16)
        nc.vector.memset(zeros_sb, 0.0)
        swp_sb = const.tile([128, 128], dt.bfloat16)
        nc.sync.dma_start(out=swp_sb[:], in_=swp_d[:])
        eps_t = const.tile([128, 1], dt.float32)
        nc.vector.memset(eps_t, EPS)

        # constant / bias tiles (one DMA for the packed affine constants)
        bias_sb = const.tile([128, 6 * NH], dt.float32)
        nc.sync.dma_start(out=bias_sb[:], in_=bias_d[:])
        bq_sb = bias_sb[:, 0 * NH:1 * NH]
        gq_sb = bias_sb[:, 1 * NH:2 * NH]
        bqgq_sb = bias_sb[:, 2 * NH:3 * NH]
        bk_sb = bias_sb[:, 3 * NH:4 * NH]
        gk_sb = bias_sb[:, 4 * NH:5 * NH]
        bkgk_sb = bias_sb[:, 5 * NH:6 * NH]
        bv_sb = const.tile([1, DIM], dt.bfloat16)
        nc.sync.dma_start(out=bv_sb[:], in_=bv_d[:])

        # x (feature-major), resident.  x and the K weights are loaded in
        # interleaved per-chunk DMAs so the first projection matmul can start
        # as soon as chunk 0 lands (a single 4.6MB weight DMA costs ~20us of
        # dead PE time at kernel start); the V/Q weight DMAs are issued early
        # in program order so they are not stuck in the DMA queue behind the
        # RoPE swap DMAs (which wait on compute).
        xT_sb = resid.tile([128, NH, S], dt.bfloat16)

        # raw RoPE sin/cos (shared q/k); the raw angles only live until the
        # Sin lookups run
        sin_raw = resid.tile([128, S], dt.float32)
        cos_raw = resid.tile([128, S], dt.float32)
        with tc.tile_pool(name="ang", bufs=1) as angpool:
            angS_sb = angpool.tile([128, S], dt.float32, name="angS_sb")
            angC_sb = angpool.tile([128, S], dt.float32, name="angC_sb")
            nc.sync.dma_start(out=angS_sb[:], in_=angS_d[:])
            nc.sync.dma_start(out=angC_sb[:], in_=angC_d[:])
            nc.scalar.activation(sin_raw[:], angS_sb[:], AF.Sin)
            nc.scalar.activation(cos_raw[:], angC_sb[:], AF.Sin)

        # k-hat is consumed by RoPE before the Q projection writes q-hat, so
        # one buffer serves both
        qhat = khat = resid.tile([128, NH, S], dt.bfloat16, name="hat")
        qrot = resid.tile([128, NH, S], dt.bfloat16)
        krot = resid.tile([128, NH, S], dt.bfloat16)
        # krot is consumed by the kv_loc writes long before the attention
        # output is produced, so o^T shares its buffer
        r_q = resid.tile([1, S], dt.float32)
        r_k = resid.tile([1, S], dt.float32)
        oT_sb = krot

        halves = [(0, H0), (H0, S)] if S > H0 else [(0, S)]

        # ---------------- Q/K projections + RMS stats ----------------
        def qk_proj(wname, w_sb, bias_sb, gain_sb, bg_sb, hat, r_sb):
          with ExitStack() as pctx:
            pspool = pctx.enter_context(
                tc.tile_pool(name=f"ps_{wname}", bufs=6, space="PSUM"))
            sspool = pctx.enter_context(
                tc.tile_pool(name=f"ss_{wname}", bufs=2, space="PSUM"))
            evpool = pctx.enter_context(tc.tile_pool(name=f"ev_{wname}", bufs=3))
            ss_ps = {}
            for hi, (ha, hb) in enumerate(halves):
                ss_ps[hi] = sspool.tile([1, hb - ha], dt.float32, tag="ss", name=f"ss{hi}")
            for m in range(NH):
                ps = {}
                for hi, (ha, hb) in enumerate(halves):
                    ps[hi] = pspool.tile([128, hb - ha], dt.float32, tag="ps", name=f"ps{hi}")
                for kc in range(NH):
                    for hi, (ha, hb) in enumerate(halves):
                        nc.tensor.matmul(ps[hi][:, :hb - ha],
                                         w_sb[:, kc, m * 128:(m + 1) * 128],
                                         xT_sb[:, kc, ha:hb],
                                         start=(kc == 0), stop=(kc == NH - 1))
                for hi, (ha, hb) in enumerate(halves):
                    hw_ = hb - ha
                    sq = evpool.tile([128, H0], dt.bfloat16, tag="sq")
                    # (q + b)^2
                    nc.scalar.activation(sq[:, :hw_], ps[hi][:, :hw_], AF.Square,
                                         bias=bias_sb[:, m:m + 1])
                    # qhat = (q + b) * g = q*g + b*g
                    nc.scalar.activation(hat[:, m, ha:hb], ps[hi][:, :hw_],
                                         AF.Identity, bias=bg_sb[:, m:m + 1],
                                         scale=gain_sb[:, m:m + 1])
                    nc.tensor.matmul(ss_ps[hi][0:1, :hw_], ones_key[:],
                                     sq[:, :hw_],
                                     start=(m == 0), stop=(m == NH - 1))
            for hi, (ha, hb) in enumerate(halves):
                hw_ = hb - ha
                rt = evpool.tile([1, H0], dt.float32, tag="rt")
                # sqrt(mean(q^2) + eps)
                nc.scalar.activation(rt[0:1, :hw_], ss_ps[hi][0:1, :hw_], AF.Sqrt,
                                     bias=eps_t[0:1, :], scale=1.0 / DIM)
                nc.vector.reciprocal(r_sb[0:1, ha:hb], rt[0:1, :hw_])

        # ---------------- RoPE ----------------
        def rope(hat, rot, r_sb, tag, pctx, sw_bufs=3):
            rp = pctx.enter_context(tc.tile_pool(name=f"rope_{tag}", bufs=3))
            rps = pctx.enter_context(
                tc.tile_pool(name=f"rps_{tag}", bufs=1, space="PSUM"))
            # broadcast r (per-token 1/rms) across partitions on the PE
            # (rank-1 matmul) -- no DRAM roundtrip
            rb = {}
            for (ba, bb) in _segs(0, S):
                rb[ba] = rps.tile([128, bb - ba], dt.float32, name=f"rb_{tag}{ba}")
                nc.tensor.matmul(rb[ba][:, :bb - ba], ones_rowf[:],
                                 r_sb[0:1, ba:bb], start=True, stop=True)
            ct = resid.tile([128, S], dt.bfloat16, name=f"cos_{tag}")
            st = resid.tile([128, S], dt.bfloat16, name=f"sin_{tag}")
            for (ba, bb) in _segs(0, S):
                nc.vector.tensor_mul(ct[:, ba:bb], cos_raw[:, ba:bb], rb[ba][:])
                nc.vector.tensor_mul(st[:, ba:bb], sin_raw[:, ba:bb], rb[ba][:])
            swps = pctx.enter_context(
                tc.tile_pool(name=f"swps_{tag}", bufs=sw_bufs, space="PSUM"))
            for m in range(NH):
                # half-swap on the PE (permutation matmul) -- an SBUF-to-SBUF
                # swap DMA would serialize against the AllGather
                sw = swps.tile([128, S], dt.float32, tag="sw")
                for (ba, bb) in _segs(0, S):
                    nc.tensor.matmul(sw[:, ba:bb], swp_sb[:],
                                     hat[:, m, ba:bb], start=True, stop=True)
                t1 = rp.tile([128, S], dt.bfloat16, tag="t1")
                t2 = rp.tile([128, S], dt.bfloat16, tag="t2")
                nc.vector.tensor_mul(t1[:], hat[:, m, :], ct[:])
                for (ba, bb) in _segs(0, S):
                    nc.vector.tensor_mul(t2[:, ba:bb], sw[:, ba:bb],
                                         st[:, ba:bb])
                nc.vector.tensor_add(rot[:, m, :], t1[:], t2[:])

        # ---------------- V projection (token-major) ----------------
        # token-chunk-outer so only NSL(+1) PSUM banks are live -- this lets
        # the V projection share PSUM with the k-RoPE pools and overlap it
        def v_proj(w_sb, pctx):
            pspool = pctx.enter_context(
                tc.tile_pool(name="ps_v", bufs=NSL + 1, space="PSUM"))
            evpool = pctx.enter_context(tc.tile_pool(name="ev_v", bufs=3))
            for ti, (ta, tb) in enumerate(TOKCH):
                tw = tb - ta
                ps = {}
                for sl in range(NSL):
                    ps[sl] = pspool.tile([128, SLICE], dt.float32, tag="vps",
                                         name=f"vps{sl}")
                for kc in range(NH):
                    for sl in range(NSL):
                        nc.tensor.matmul(ps[sl][:tw, :], xT_sb[:, kc, ta:tb],
                                         w_sb[:, kc, sl * SLICE:(sl + 1) * SLICE],
                                         start=(kc == 0), stop=False)
                for sl in range(NSL):
                    nc.tensor.matmul(ps[sl][:tw, :], ones_row[0:1, :tw],
                                     bv_sb[0:1, sl * SLICE:(sl + 1) * SLICE],
                                     start=False, stop=True)
                    vt = evpool.tile([128, SLICE], dt.bfloat16, tag="vev")
                    nc.scalar.activation(vt[:tw, :], ps[sl][:tw, :], AF.Copy)
                    nc.sync.dma_start(
                        out=bass.AP(
                            tensor=kv_loc.tensor,
                            offset=kv_base + DIM * S + ta * DIM + sl * SLICE,
                            ap=[[DIM, tw], [1, SLICE]]),
                        in_=vt[:tw, :])

        # ---- phase order: K then V (fills kv_loc), AllGather, then Q ----
        # The collective runs on its own lane; the Q projection + RoPE have no
        # dependency on it and overlap its latency.
        wk_ctx = ExitStack()
        wkpool = wk_ctx.enter_context(tc.tile_pool(name="w_wk", bufs=1))
        wv_ctx = ExitStack()
        wvpool = wv_ctx.enter_context(tc.tile_pool(name="w_wv", bufs=1))
        wk_sb = wkpool.tile([128, NH, DIM], dt.bfloat16, name="wsb_k",
                            tag="wkq")
        wv_sb = wvpool.tile([128, NH, DIM], dt.bfloat16, name="wsb_v")
        for kc in range(NH):
            nc.sync.dma_start(out=wk_sb[:, kc, :],
                              in_=w_d["wkT"][kc * 128:(kc + 1) * 128, :])
            nc.sync.dma_start(out=xT_sb[:, kc, :],
                              in_=xT_d[kc * 128:(kc + 1) * 128, :])
        nc.sync.dma_start(
            out=wv_sb[:], in_=w_d["wvT"][:].rearrange("(kc p) n -> p kc n", p=128))

        qk_proj("wkT", wk_sb, bk_sb, gk_sb, bkgk_sb, khat, r_k)
        # Q weights reuse the K-weight pool slot (freed once the K projection
        # is done reading it)
        wq_sb = wkpool.tile([128, NH, DIM], dt.bfloat16, name="wsb_q",
                            tag="wkq")
        nc.sync.dma_start(
            out=wq_sb[:], in_=w_d["wqT"][:].rearrange("(kc p) n -> p kc n", p=128))
        # k-RoPE and the V projection share one scope (PSUM 4+4 banks) so the
        # scheduler can overlap them instead of serializing on pool reuse
        with ExitStack() as kvctx:
            rope(khat, krot, r_k, "k", kvctx, sw_bufs=1)
            for m in range(NH):
                nc.sync.dma_start(out=kv_loc[m * 128:(m + 1) * 128, :],
                                  in_=krot[:, m, :])
            v_proj(wv_sb, kvctx)
        wv_ctx.close()
        nc.gpsimd.collective_compute(
            "AllGather", mybir.AluOpType.bypass, ins=[kv_loc[:]],
            outs=[kv_all[:]], replica_groups=rg)
        kr0 = stage_kr(0)

        qk_proj("wqT", wq_sb, bq_sb, gq_sb, bqgq_sb, qhat, r_q)
        wk_ctx.close()

        # ---------------- attention ----------------
        # The softmax denominator Z is accumulated in partition-broadcast
        # form ([128, qw] ones-matmuls mirroring the o accumulation), so 1/Z
        # is a direct DVE reciprocal -- no DRAM roundtrip.
        actx = ctx.enter_context(ExitStack())
        att_v = actx.enter_context(tc.tile_pool(name="att_v", bufs=18))

        # keys of one frame, flattened across cores: g = c*T + t, chunked in
        # groups of <=120 so every score/PV matmul runs a full-ish chunk
        FRAME_K = NC * T
        KW = 120
        NTI = (FRAME_K + KW - 1) // KW
        kchunks = []           # per chunk: list of (row0, c, t0, len) spans
        for ti in range(NTI):
            g, end = ti * KW, min((ti + 1) * KW, FRAME_K)
            spans = []
            while g < end:
                c, t = g // T, g % T
                take = min(end - g, T - t)
                spans.append((g - ti * KW, c, t, take))
                g += take
            kchunks.append(spans)
        QA = min(512, S)       # A region: queries [0, QA); B: [QA, S)
        WB = S - QA
        assert 0 < WB <= 128
        # A-region score-tile column layout: one block per key frame
        q0f = [T * first_qf[kf] for kf in range(F)]
        colA = []
        acc = 0
        for kf in range(F):
            colA.append(acc)
            acc += max(0, QA - q0f[kf])
        WA = acc               # total A-tile width (<= 1024)
        assert WA <= 1024

        def stage_v(pg):
            out = {}
            for ti in range(NTI):
                out[ti] = att_v.tile([128, F, 256], dt.bfloat16,
                                     tag="v", name=f"v{ti}")
                for (r0, c, t0, ln) in kchunks[ti]:
                    nc.sync.dma_start(
                        out=out[ti][r0:r0 + ln, :, :],
                        in_=bass.AP(
                            tensor=kv_all.tensor,
                            offset=kva_base + (c * 2 * DIM + DIM) * S
                            + pg * 256 + t0 * DIM,
                            ap=[[DIM, ln], [T * DIM, F], [1, 256]]))
            return out

        # pg0's V staging issues ahead of q-RoPE so it follows the K gathers
        # in the DMA queue
        v0 = stage_v(0)
        with ExitStack() as qctx:
            rope(qhat, qrot, r_q, "q", qctx)

        att_s = actx.enter_context(tc.tile_pool(name="att_s", bufs=2, space="PSUM"))
        att_sb = actx.enter_context(
            tc.tile_pool(name="att_sb", bufs=1, space="PSUM"))
        att_o = actx.enter_context(tc.tile_pool(name="att_o", bufs=1, space="PSUM"))
        att_p = actx.enter_context(tc.tile_pool(name="att_p", bufs=6))
        att_m = actx.enter_context(tc.tile_pool(name="att_m", bufs=2))

        NPG = NH // 2
        for pg in range(NPG):
            # stage this head-pair's K and V tiles.  K: one gather per frame
            # pulling all 8 cores' slices into a core-flattened key axis.
            # V: one tile per key chunk (split where the chunk crosses a
            # core boundary).
            if pg == 0:
                kr_t, v_t = kr0, v0
            else:
                kr_t, v_t = stage_kr(pg), stage_v(pg)
            for hi in range(2):
                h = 2 * pg + hi
                o_A = att_o.tile([128, QA], dt.float32, tag="oA", name="oA")
                z_A = att_o.tile([128, QA], dt.float32, tag="zA", name="zA")
                # packed bank for the two B regions (o at 0, z at 128):
                # zero once, accumulate start=False, close on the final z
                pk = att_o.tile([128, 256], dt.float32, tag="oB", name="oB")
                nc.tensor.matmul(pk[:, :], ones128[:], zeros_sb[:, :256],
                                 start=True, stop=False, skip_group_check=True)
                # running sums of the exp tiles (for Z) live on the DVE, so
                # the softmax denominator costs 6 matmuls/head instead of 96
                ps_A = att_m.tile([128, 1024], dt.bfloat16, tag="psA",
                                  name="psA")
                ps_B = att_m.tile([128, WB * F], dt.bfloat16, tag="psB",
                                  name="psB")
                for ti in range(NTI):
                    kw = min(KW, FRAME_K - ti * KW)
                    g0 = ti * KW
                    s_A = att_s.tile([128, 1024], dt.float32, tag="sA")
                    s_B = att_sb.tile([128, WB * F], dt.float32, tag="sB")
                    for kf in range(F):
                        ksl = kr_t[kf][:, hi, g0:g0 + kw]
                        wa = QA - q0f[kf]
                        if wa > 0:
                            nc.tensor.matmul(
                                s_A[:kw, colA[kf]:colA[kf] + wa], ksl,
                                qrot[:, h, q0f[kf]:QA],
                                start=True, stop=True)
                        nc.tensor.matmul(
                            s_B[:kw, kf * WB:(kf + 1) * WB], ksl,
                            qrot[:, h, QA:S], start=True, stop=True)
                    p_A = att_p.tile([128, 1024], dt.bfloat16, tag="p")
                    nc.scalar.activation(p_A[:kw, :WA], s_A[:kw, :WA],
                                         AF.Exp, scale=inv_sqrt_d)
                    p_B = att_p.tile([128, WB * F], dt.bfloat16, tag="pb")
                    nc.scalar.activation(p_B[:kw, :], s_B[:kw, :],
                                         AF.Exp, scale=inv_sqrt_d)
                    first = ti == 0
                    last = ti == NTI - 1
                    if first:
                        nc.vector.tensor_copy(ps_A[:kw, :WA], p_A[:kw, :WA])
                        nc.vector.tensor_copy(ps_B[:kw, :], p_B[:kw, :])
                    else:
                        nc.vector.tensor_add(ps_A[:kw, :WA], ps_A[:kw, :WA],
                                             p_A[:kw, :WA])
                        nc.vector.tensor_add(ps_B[:kw, :], ps_B[:kw, :],
                                             p_B[:kw, :])
                    for kf in range(F):
                        vsl = v_t[ti][:kw, kf, hi * 128:(hi + 1) * 128]
                        wa = QA - q0f[kf]
                        if wa > 0:
                            nc.tensor.matmul(
                                o_A[:, q0f[kf]:QA], vsl,
                                p_A[:kw, colA[kf]:colA[kf] + wa],
                                start=first and kf == 0,
                                stop=last and kf == F - 1)
                        nc.tensor.matmul(
                            pk[:, 0:WB], vsl,
                            p_B[:kw, kf * WB:(kf + 1) * WB],
                            start=False, stop=False,
                            skip_group_check=True)
                # Z from the p sums: one ones-matmul per kf block
                for kf in range(F):
                    wa = QA - q0f[kf]
                    if wa > 0:
                        nc.tensor.matmul(
                            z_A[:, q0f[kf]:QA], ones128[:KW, :],
                            ps_A[:KW, colA[kf]:colA[kf] + wa],
                            start=kf == 0, stop=kf == F - 1)
                    nc.tensor.matmul(
                        pk[:, 128:128 + WB], ones128[:KW, :],
                        ps_B[:KW, kf * WB:(kf + 1) * WB],
                        start=False, stop=kf == F - 1,
                        skip_group_check=True)
                # 1/Z and eviction for head h
                izb = att_m.tile([128, S], dt.float32, tag="izb", name="izb")
                nc.vector.reciprocal(izb[:, 0:QA], z_A[:, :])
                nc.vector.reciprocal(izb[:, QA:S], pk[:, 128:128 + WB])
                nc.vector.tensor_mul(oT_sb[:, h, 0:QA], o_A[:, :],
                                     izb[:, 0:QA])
                nc.vector.tensor_mul(oT_sb[:, h, QA:S], pk[:, 0:WB],
                                     izb[:, QA:S])

        actx.close()  # release attention PSUM banks before the O-projection

        # ---------------- O projection ----------------
        wpool = ctx.enter_context(tc.tile_pool(name="w_o", bufs=2))
        obias = ctx.enter_context(tc.tile_pool(name="obias", bufs=1))
        bo_bc = obias.tile([128, DIM], dt.float32)
        nc.sync.dma_start(
            out=bo_bc[:],
            in_=bass.AP(tensor=bo_d[:].tensor, offset=bo_d[:].offset,
                        ap=[[0, 128]] + bo_d[:].ap[1:]),
        )
        pspool = ctx.enter_context(
            tc.tile_pool(name="ps_o", bufs=len(TOKCH) + 1, space="PSUM"))
        evpool = ctx.enter_context(tc.tile_pool(name="ev_o", bufs=3))
        for sl in range(NSL):
            wt = wpool.tile([128, NH, SLICE], dt.bfloat16, tag="wo")
            nc.sync.dma_start(
                out=wt[:],
                in_=w_d["woT"][:, sl * SLICE:(sl + 1) * SLICE]
                .rearrange("(m p) n -> p m n", p=128))
            ps = {}
            for ti in range(len(TOKCH)):
                ps[ti] = pspool.tile([128, SLICE], dt.float32, tag="ops", name=f"ops{ti}")
            for m in range(NH):
                for ti, (ta, tb) in enumerate(TOKCH):
                    nc.tensor.matmul(ps[ti][:tb - ta, :], oT_sb[:, m, ta:tb],
                                     wt[:, m, :], start=(m == 0), stop=(m == NH - 1))
            for ti, (ta, tb) in enumerate(TOKCH):
                tw = tb - ta
                ot = evpool.tile([128, SLICE], dt.float32, tag="oev")
                nc.vector.tensor_add(ot[:tw, :], ps[ti][:tw, :],
                                     bo_bc[:tw, sl * SLICE:(sl + 1) * SLICE])
                nc.sync.dma_start(
                    out=out_d[ta:tb, sl * SLICE:(sl + 1) * SLICE],
                    in_=ot[:tw, :])

    if cap_waits:
        _cap_sync_waits(nc, mybir)
    _BUILD_CACHE[key] = nc
    return nc


def _cap_sync_waits(nc, mybir, cap=1):
    """Walrus engine-instruction structs only have a limited number of sync
    wait slots.  Hoist excess waits onto InstNoOp carriers placed immediately
    before the instruction on the same engine stream."""
    exempt = (mybir.InstNoOp, mybir.InstEventSemaphore,
              mybir.InstAllEngineBarrier)
    for f in nc.m.functions:
        for bb in f.blocks:
            out = []
            changed = False
            for inst in bb.instructions:
                si = inst.sync_info
                if (si is None or len(si.on_wait) <= cap
                        or isinstance(inst, exempt)):
                    out.append(inst)
                    continue
                waits = list(si.on_wait)
                keep, excess = waits[:cap], waits[cap:]
                while excess:
                    batch, excess = excess[:cap], excess[cap:]
                    out.append(mybir.InstNoOp(
                        name=f"{inst.name}-w{len(out)}",
                        engine=inst.engine,
                        bass_nofuse=True,
                        sync_info=mybir.SyncInfo(on_wait=batch, on_update=[]),
                    ))
                inst.sync_info = mybir.SyncInfo(on_wait=keep,
                                                on_update=list(si.on_update))
                out.append(inst)
                changed = True
            if changed:
                bb.instructions = out


# ---------------------------------------------------------------------------
# host side
# ---------------------------------------------------------------------------
def _perm(NH):
    p = np.empty(NH * D, np.int64)
    for hh in range(NH):
        base = hh * D
        for j in range(D // 2):
            p[base + j] = base + 2 * j
            p[base + D // 2 + j] = base + 2 * j + 1
    return p


def _host_inputs(x, freqs, Wq, bq, Wk, bk, Wv, bv, Wo, bo, gq, gk,
                 f, h, w, num_heads, local_attn_size, sink_size, start_frame):
    NH = num_heads
    DIM = NH * D
    FRAME = h * w
    assert FRAME % NC == 0
    T = FRAME // NC
    S = f * T
    perm = _perm(NH)

    def bf(a):
        return np.ascontiguousarray(a, dtype=np.float32).astype(BF16)

    wqT = bf(Wq[perm].T)
    wkT = bf(Wk[perm].T)
    wvT = bf(Wv.T)
    woT = bf(Wo.T)
    def chunkmajor(a):
        return np.asarray(a, np.float32)[perm].reshape(NH, D).T
    bias_pack = np.ascontiguousarray(np.concatenate(
        [chunkmajor(bq), chunkmajor(gq), chunkmajor(bq) * chunkmajor(gq),
         chunkmajor(bk), chunkmajor(gk), chunkmajor(bk) * chunkmajor(gk)],
        axis=1), np.float32)
    bv_r = bf(bv.reshape(1, DIM))
    bo_r = np.ascontiguousarray(bo.reshape(1, DIM), np.float32)
    swp = np.zeros((128, 128), np.float32)
    swp[(np.arange(128) + 64) % 128, np.arange(128)] = 1.0
    swp = swp.astype(BF16)

    c = D // 2
    c1 = c // 3
    c0 = c - 2 * c1
    freqs = np.asarray(freqs, np.float32)

    in_maps = []
    tok_idx = []
    for core in range(NC):
        idx = np.concatenate(
            [fr * FRAME + T * core + np.arange(T) for fr in range(f)])
        tok_idx.append(idx)
        xT = bf(np.asarray(x[0], np.float32)[idx].T)
        fr = idx // FRAME
        rem = idx % FRAME
        hh_i = rem // w
        ww_i = rem % w
        ang = np.empty((c, S), np.float32)
        ang[:c0, :] = freqs[start_frame + fr][:, :c0].T
        ang[c0:c0 + c1, :] = freqs[hh_i][:, c0:c0 + c1].T
        ang[c0 + c1:, :] = freqs[ww_i][:, c0 + c1:c].T
        def wrap(a):
            a = np.asarray(a, np.float64)
            return (a - 2 * np.pi * np.round(a / (2 * np.pi))).astype(np.float32)
        # top half encodes -sin via the (ang + pi) phase shift
        angS = np.ascontiguousarray(
            np.concatenate([wrap(ang + np.pi), wrap(ang)], 0), np.float32)
        angC = np.ascontiguousarray(
            np.concatenate([wrap(ang + np.pi / 2), wrap(ang + np.pi / 2)], 0),
            np.float32)
        in_maps.append({
            "xT": xT, "wqT": wqT, "wkT": wkT, "wvT": wvT, "woT": woT,
            "bias_pack": bias_pack, "swp": swp,
            "bv_r": bv_r, "bo_r": bo_r, "angS": angS, "angC": angC,
        })
    return in_maps, tok_idx, T, S


def _allowed(f, local_attn_size, sink_size):
    return [
        [kf for kf in range(f)
         if kf <= qf and (qf - kf < local_attn_size or kf < sink_size)]
        for qf in range(f)
    ]


def kernel(x, freqs, Wq, bq, Wk, bk, Wv, bv, Wo, bo, gq, gk,
           f, h, w, num_heads, local_attn_size, sink_size, start_frame,
           _trace=False):
    from concourse.bass_utils import run_bass_kernel_spmd

    f = int(f); h = int(h); w = int(w)
    num_heads = int(num_heads)
    local_attn_size = int(local_attn_size)
    sink_size = int(sink_size)
    start_frame = int(start_frame)

    x = np.asarray(x)
    B, L, DIM = x.shape
    assert B == 1 and DIM == num_heads * D

    allowed = _allowed(f, local_attn_size, sink_size)
    in_maps, tok_idx, T, S = _host_inputs(
        x, freqs, Wq, bq, Wk, bk, Wv, bv, Wo, bo, gq, gk,
        f, h, w, num_heads, local_attn_size, sink_size, start_frame)
    nc = build_program(num_heads, f, T, allowed)
    res = run_bass_kernel_spmd(nc, in_maps, core_ids=list(range(NC)),
                               trace=_trace)
    out = np.empty((1, L, DIM), np.float32)
    for core in range(NC):
        out[0, tok_idx[core]] = res.results[core]["out"]
    if _trace:
        kernel._last_results = res
    return out

